# revision 1
# baseline (speedup 1.0000x reference)
"""CondGraphConv Trainium2 kernel: 8-core SPMD, i-sharded edges.

Algebraic restructuring:
    x_e  = Ci[i_e] + Cj[j_e] + relu(sp_e @ Ws + bs) @ Wl_s
    out_e = relu(LN(x_e) * gamma[bid[j_e]] + beta[bid[j_e]])
  where Ci = h @ Wl[:128], Cj = h @ Wl[128:256], h = relu(nf @ Wn + bn).

Sharding: core c owns edges with i in [800c, 800c+800), processed sorted by
i.  Indices are known at program-build time, so the i-side gather Ci[i_e]
compiles into static one-hot matmuls (host-baked selector) against an
SBUF-resident local Ci window; the j-side (random) uses one indirect DMA per
128-edge tile fetching combined 768B rows Tj = [Cj | gamma' | beta].
gamma' = gamma + 1 and all biases are folded in on the host.  Output rows are
written in schedule order; the host inverse-permutes.
"""

import sys
import types

for _p in ("/opt/trn_rl_repo",):
    if _p not in sys.path:
        sys.path.append(_p)

import numpy as np

N, E, B = 6400, 313600, 128
NODE_DIM, COND_DIM, EDGE_DIM = 2048, 1024, 128
S_IN, S_OUT = 8, 30
EPS = 1e-5

NCORES = 8
NLOC = N // NCORES            # 800 own i-nodes per core
NLB = (NLOC + 127) // 128     # 7 local blocks
TILE = 128
F16 = np.float16

_cache = {}


def _axon_shim():
    try:
        import antenv.axon_hooks  # noqa: F401
        return
    except ImportError:
        pass
    try:
        import antenv
        from trn_agent_boot.trn_boot import _ntff_profile_via_ctypes
    except ImportError:
        return
    mod = types.ModuleType("antenv.axon_hooks")
    holder = [None]
    mod.set_axon_ntff_profile_hook = lambda h: holder.__setitem__(0, h)
    mod.get_axon_ntff_profile_hook = lambda: holder[0]
    sys.modules["antenv.axon_hooks"] = mod
    antenv.axon_hooks = mod
    try:
        mod.set_axon_ntff_profile_hook(
            _ntff_profile_via_ctypes("/opt/axon/libaxon_pjrt.so")
        )
    except Exception:
        pass


def _plan(inputs):
    """Shard by i-range, sort by i, tile within local 128-node blocks, build
    the core-uniform schedule (list of local block ids, one per tile)."""
    ii = np.asarray(inputs["node_i_ids"]).astype(np.int64)
    jj = np.asarray(inputs["node_j_ids"]).astype(np.int64)

    plans = []
    counts = np.zeros((NCORES, NLB), np.int64)
    for c in range(NCORES):
        lo = c * NLOC
        eids = np.nonzero((ii >= lo) & (ii < lo + NLOC))[0]
        order = np.argsort(ii[eids], kind="stable")
        eids = eids[order]
        il = ii[eids] - lo                       # local i in [0, 800)
        lb = il // 128                           # local block 0..6
        # tiles: within each local block, groups of <=128 edges
        tiles = []                               # (start, cnt, lblock)
        s = 0
        ne = eids.shape[0]
        while s < ne:
            b = lb[s]
            e = min(s + TILE, ne)
            e = s + int(np.searchsorted(lb[s:e], b + 1))
            tiles.append((s, e - s, int(b)))
            counts[c, int(b)] += 1
            s = e
        plans.append({"eids": eids, "il": il, "jv": jj[eids], "tiles": tiles})

    maxcnt = counts.max(axis=0)
    sched = []
    for b in range(NLB):
        sched.extend([b] * int(maxcnt[b]))
    # pad to multiple of 4 (out-batching) with block-0 dummy tiles
    while len(sched) % 4:
        sched.append(0)
    return plans, sched


def _prep_inputs(inputs, plans, sched):
    KC_H = NODE_DIM // 128
    KC_GB = COND_DIM // 128 + 1
    KDIM_GB = KC_GB * 128
    nt = len(sched)

    nf = np.asarray(inputs["node_feats"], np.float32)
    nfT = np.ascontiguousarray(nf.T.astype(F16))
    wnA = np.asarray(inputs["Wn"], np.float32).astype(F16)
    bnc = np.asarray(inputs["bn"], np.float32).reshape(128, 1)

    cond = np.asarray(inputs["cond_feats"], np.float32)
    condA = np.zeros((KDIM_GB, B), F16)
    condA[:COND_DIM] = cond.T.astype(F16)
    condA[COND_DIM] = 1.0
    wcA = np.zeros((KDIM_GB, 256), F16)
    wcA[:COND_DIM] = np.asarray(inputs["Wc"], np.float32).astype(F16)
    bc_plus = np.asarray(inputs["bc"], np.float32).copy()
    bc_plus[:EDGE_DIM] += 1.0
    wcA[COND_DIM] = bc_plus.astype(F16)

    ws = np.asarray(inputs["Ws"], np.float32).astype(F16)
    bs = np.asarray(inputs["bs"], np.float32).reshape(S_OUT, 1)
    wl = np.asarray(inputs["Wl"], np.float32).astype(F16)

    bid = np.asarray(inputs["batch_ids"]).astype(np.int64)
    ohB = np.zeros((N // 128, 128, 128), F16)    # [blk, batch, node]
    for blk in range(N // 128):
        nb = bid[blk * 128:(blk + 1) * 128]
        ohB[blk, nb, np.arange(128)] = 1.0

    spT_full = np.asarray(inputs["spatial_feats"], np.float32).T.astype(F16)

    shared = dict(
        nfT=nfT, wnA=wnA, condA=condA, wcA=wcA, ws=ws, bs=bs, bnc=bnc,
        wlhi=wl[:128].copy(), wlhj=wl[128:256].copy(), wls=wl[256:].copy(),
        ohB=ohB,
    )
    in_maps = []
    for c, p in enumerate(plans):
        # distribute this core's tiles into the uniform schedule slots
        slot_of_block = {}
        for t, b in enumerate(sched):
            slot_of_block.setdefault(b, []).append(t)
        used = {b: 0 for b in range(NLB)}
        ohI = np.zeros((nt, 128, 128), F16)      # [t, window_row, slot]
        idxJ = np.zeros((128, nt), np.int32)
        spc = np.zeros((S_IN, nt * TILE), F16)
        slotmap = np.full(nt * TILE, -1, np.int64)   # slot -> original edge id
        for (s, cnt, b) in p["tiles"]:
            t = slot_of_block[b][used[b]]
            used[b] += 1
            rows = p["il"][s:s + cnt] - b * 128
            ohI[t, rows, np.arange(cnt)] = 1.0
            idxJ[:cnt, t] = p["jv"][s:s + cnt]
            spc[:, t * TILE:t * TILE + cnt] = spT_full[:, p["eids"][s:s + cnt]]
            slotmap[t * TILE:t * TILE + cnt] = p["eids"][s:s + cnt]
        # Ci local window offsets: global rows 800c + 128b + p (clamped)
        ciwin = np.zeros((128, NLB), np.int32)
        for b in range(NLB):
            g = c * NLOC + b * 128 + np.arange(128)
            ciwin[:, b] = np.minimum(g, N - 1)
        m = dict(shared)
        m["ohI"] = ohI
        m["idxJ"] = idxJ
        m["spc"] = spc
        m["ciwin"] = ciwin
        in_maps.append(m)
        p["slotmap"] = slotmap
    return in_maps


def _build_program(sched):
    import concourse.bass as bass
    import concourse.tile as tile
    from concourse import bacc, mybir
    from contextlib import ExitStack

    f16 = mybir.dt.float16
    f32 = mybir.dt.float32
    i32 = mybir.dt.int32
    AF = mybir.ActivationFunctionType
    OP = mybir.AluOpType

    KC_H = NODE_DIM // 128
    KDIM_H = NODE_DIM
    KC_GB = COND_DIM // 128 + 1
    NBLK = N // 128
    NB1 = 512
    NCH1 = (N + NB1 - 1) // NB1
    nt = len(sched)

    nc = bacc.Bacc(
        "TRN2", target_bir_lowering=False, debug=False,
        num_devices=NCORES, num_swdge_queues=1,
    )

    nfT = nc.dram_tensor("nfT", [KDIM_H, N], f16, kind="ExternalInput")
    wnA = nc.dram_tensor("wnA", [KDIM_H, 128], f16, kind="ExternalInput")
    condA = nc.dram_tensor("condA", [KC_GB * 128, B], f16, kind="ExternalInput")
    wcA = nc.dram_tensor("wcA", [KC_GB * 128, 256], f16, kind="ExternalInput")
    ws = nc.dram_tensor("ws", [S_IN, S_OUT], f16, kind="ExternalInput")
    bs = nc.dram_tensor("bs", [S_OUT, 1], f32, kind="ExternalInput")
    bnc = nc.dram_tensor("bnc", [128, 1], f32, kind="ExternalInput")
    wlhi = nc.dram_tensor("wlhi", [128, 128], f16, kind="ExternalInput")
    wlhj = nc.dram_tensor("wlhj", [128, 128], f16, kind="ExternalInput")
    wls = nc.dram_tensor("wls", [S_OUT, 128], f16, kind="ExternalInput")
    ohB = nc.dram_tensor("ohB", [NBLK, 128, 128], f16, kind="ExternalInput")
    ohI = nc.dram_tensor("ohI", [nt, 128, 128], f16, kind="ExternalInput")
    idxJ = nc.dram_tensor("idxJ", [128, nt], i32, kind="ExternalInput")
    spc = nc.dram_tensor("spc", [S_IN, nt * TILE], f16, kind="ExternalInput")
    ciwin = nc.dram_tensor("ciwin", [128, NLB], i32, kind="ExternalInput")
    out = nc.dram_tensor("out", [nt * TILE, 128], f32, kind="ExternalOutput")

    ci_dram = nc.dram_tensor("ci_tbl", [N, 128], f16)
    tj_dram = nc.dram_tensor("tj_tbl", [N, 384], f16)

    with tile.TileContext(nc) as tc:
        with ExitStack() as ctx:
            const = ctx.enter_context(tc.tile_pool(name="const", bufs=1))

            wn_sb = const.tile([128, KC_H * 128], f16)
            for k in range(KC_H):
                nc.sync.dma_start(
                    out=wn_sb[:, k * 128:(k + 1) * 128],
                    in_=wnA.ap()[k * 128:(k + 1) * 128, :],
                )
            cond_sb = const.tile([128, KC_GB * 128], f16)
            wc_sb = const.tile([128, KC_GB * 256], f16)
            for k in range(KC_GB):
                nc.sync.dma_start(
                    out=cond_sb[:, k * 128:(k + 1) * 128],
                    in_=condA.ap()[k * 128:(k + 1) * 128, :],
                )
                nc.sync.dma_start(
                    out=wc_sb[:, k * 256:(k + 1) * 256],
                    in_=wcA.ap()[k * 128:(k + 1) * 128, :],
                )
            ws_sb = const.tile([S_IN, S_OUT], f16)
            nc.sync.dma_start(out=ws_sb[:], in_=ws.ap())
            bs_sb = const.tile([S_OUT, 1], f32)
            nc.sync.dma_start(out=bs_sb[:], in_=bs.ap())
            bn_sb = const.tile([128, 1], f32)
            nc.sync.dma_start(out=bn_sb[:], in_=bnc.ap())
            wlhi_sb = const.tile([128, 128], f16)
            nc.sync.dma_start(out=wlhi_sb[:], in_=wlhi.ap())
            wlhj_sb = const.tile([128, 128], f16)
            nc.sync.dma_start(out=wlhj_sb[:], in_=wlhj.ap())
            wls_sb = const.tile([S_OUT, 128], f16)
            nc.sync.dma_start(out=wls_sb[:], in_=wls.ap())
            idxj_sb = const.tile([128, nt], i32)
            nc.sync.dma_start(out=idxj_sb[:], in_=idxJ.ap())
            ciwin_sb = const.tile([128, NLB], i32)
            nc.sync.dma_start(out=ciwin_sb[:], in_=ciwin.ap())
            eps_sb = const.tile([128, 1], f32)
            nc.vector.memset(eps_sb[:], EPS)
            ci_loc = const.tile([128, NLB, 128], f16)

            # ================= phase 1: node tables =================
            with ExitStack() as p1:
                w1 = p1.enter_context(tc.tile_pool(name="w1", bufs=2))
                ps1 = p1.enter_context(
                    tc.tile_pool(name="ps1", bufs=1, space="PSUM")
                )
                psT = p1.enter_context(
                    tc.tile_pool(name="psT", bufs=2, space="PSUM")
                )

                # gb' = condA.T @ wcA -> [B, 256] f16 (kept in SBUF)
                gb_ps = ps1.tile([128, 256], f32, tag="gbps")
                for k in range(KC_GB):
                    nc.tensor.matmul(
                        out=gb_ps[:],
                        lhsT=cond_sb[:, k * 128:(k + 1) * 128],
                        rhs=wc_sb[:, k * 256:(k + 1) * 256],
                        start=(k == 0), stop=(k == KC_GB - 1),
                    )
                gb_sb = const.tile([128, 256], f16)
                nc.scalar.copy(gb_sb[:], gb_ps[:])

                ohb_sb_all = const.tile([128, NBLK, 128], f16)
                nc.sync.dma_start(
                    out=ohb_sb_all[:],
                    in_=ohB.ap().rearrange("a p n -> p a n"),
                )

                for nb in range(NCH1):
                    n0 = nb * NB1
                    nsz = min(NB1, N - n0)
                    nts = nsz // 128
                    ht_psA = ps1.tile([128, NB1], f32, tag="htpsA", bufs=2)
                    ht_psB = ps1.tile([128, NB1], f32, tag="htpsB", bufs=2)
                    for k4 in range(KC_H // 4):
                        nf_t = w1.tile([128, 4, NB1], f16, tag="nft", bufs=4)
                        nc.sync.dma_start(
                            out=nf_t[:, :, :nsz],
                            in_=nfT.ap()[
                                k4 * 512:(k4 + 1) * 512, n0:n0 + nsz
                            ].rearrange("(a p) n -> p a n", p=128),
                        )
                        for kk in range(4):
                            k = k4 * 4 + kk
                            ps = ht_psA if k % 2 == 0 else ht_psB
                            nc.tensor.matmul(
                                out=ps[:, :nsz],
                                lhsT=wn_sb[:, k * 128:(k + 1) * 128],
                                rhs=nf_t[:, kk, :nsz],
                                start=(k < 2), stop=(k >= KC_H - 2),
                            )
                    ht_b = w1.tile([128, NB1], f32, tag="htb", bufs=2)
                    nc.vector.tensor_copy(ht_b[:, :nsz], ht_psB[:, :nsz])
                    ht_f = w1.tile([128, NB1], f32, tag="htf", bufs=2)
                    nc.vector.tensor_tensor(
                        out=ht_f[:, :nsz], in0=ht_psA[:, :nsz],
                        in1=ht_b[:, :nsz], op=OP.add,
                    )
                    ht_sb = w1.tile([128, NB1], f16, tag="htsb", bufs=3)
                    nc.scalar.activation(
                        ht_sb[:, :nsz], ht_f[:, :nsz], AF.Relu, bias=bn_sb[:]
                    )
                    for st in range(nts):
                        blk = (n0 + st * 128) // 128
                        lhs = ht_sb[:, st * 128:(st + 1) * 128]
                        ci_ps = ps1.tile([128, 128], f32, tag="cips")
                        nc.tensor.matmul(
                            out=ci_ps[:], lhsT=lhs, rhs=wlhi_sb[:],
                            start=True, stop=True,
                        )
                        ci_sb = w1.tile([128, 128], f16, tag="cisb", bufs=3)
                        nc.scalar.copy(ci_sb[:], ci_ps[:])
                        nc.sync.dma_start(
                            out=ci_dram.ap()[blk * 128:(blk + 1) * 128, :],
                            in_=ci_sb[:],
                        )
                        tj_sb = w1.tile([128, 384], f16, tag="tjsb", bufs=3)
                        cj_ps = ps1.tile([128, 128], f32, tag="cjps")
                        nc.tensor.matmul(
                            out=cj_ps[:], lhsT=lhs, rhs=wlhj_sb[:],
                            start=True, stop=True,
                        )
                        nc.scalar.copy(tj_sb[:, 0:128], cj_ps[:])
                        gbn_ps = ps1.tile([128, 256], f32, tag="gbnps")
                        nc.tensor.matmul(
                            out=gbn_ps[:],
                            lhsT=ohb_sb_all[:, blk, :],
                            rhs=gb_sb[:],
                            start=True, stop=True,
                        )
                        nc.vector.tensor_copy(tj_sb[:, 128:384], gbn_ps[:])
                        nc.sync.dma_start(
                            out=tj_dram.ap()[blk * 128:(blk + 1) * 128, :],
                            in_=tj_sb[:],
                        )

                tc.strict_bb_all_engine_barrier()

                # own Ci window -> SBUF via 7 indirect DMAs
                for b in range(NLB):
                    nc.gpsimd.indirect_dma_start(
                        out=ci_loc[:, b, :], out_offset=None,
                        in_=ci_dram.ap(),
                        in_offset=bass.IndirectOffsetOnAxis(
                            ap=ciwin_sb[:, b:b + 1], axis=0
                        ),
                    )

                tc.strict_bb_all_engine_barrier()

            # ================= phase 2: edges =================
            with ExitStack() as p2:
                w2 = p2.enter_context(tc.tile_pool(name="w2", bufs=3))
                oh2 = p2.enter_context(tc.tile_pool(name="oh2", bufs=2))
                sm = p2.enter_context(tc.tile_pool(name="sm", bufs=8))
                gpo = p2.enter_context(tc.tile_pool(name="gpo", bufs=24))
                ps_x = p2.enter_context(
                    tc.tile_pool(name="psx", bufs=6, space="PSUM")
                )
                ps_s = p2.enter_context(
                    tc.tile_pool(name="pss", bufs=2, space="PSUM")
                )

                OHC = 8  # one-hot tiles per prefetch chunk
                ob = None
                for t, lblk in enumerate(sched):
                    if t % OHC == 0:
                        ohc = oh2.tile([128, OHC, 128], f16, tag="ohc")
                        hi = min(OHC, nt - t)
                        nc.sync.dma_start(
                            out=ohc[:, :hi, :],
                            in_=ohI.ap()[t:t + hi].rearrange("a p n -> p a n"),
                        )
                        spt = oh2.tile([S_IN, OHC * TILE], f16, tag="spt")
                        nc.sync.dma_start(
                            out=spt[:, :hi * TILE],
                            in_=spc.ap()[:, t * TILE:(t + hi) * TILE],
                        )

                    cje = gpo.tile([128, 384], f16, tag="cje")
                    nc.gpsimd.indirect_dma_start(
                        out=cje[:], out_offset=None,
                        in_=tj_dram.ap(),
                        in_offset=bass.IndirectOffsetOnAxis(
                            ap=idxj_sb[:, t:t + 1], axis=0
                        ),
                    )

                    s_ps = ps_s.tile([S_OUT, 128], f32, tag="sps")
                    nc.tensor.matmul(
                        out=s_ps[:], lhsT=ws_sb[:],
                        rhs=spt[:, (t % OHC) * TILE:(t % OHC + 1) * TILE],
                        start=True, stop=True,
                    )
                    sT = sm.tile([S_OUT, 128], f16, tag="sT")
                    nc.scalar.activation(sT[:], s_ps[:], AF.Relu, bias=bs_sb[:])

                    x_ps = ps_x.tile([128, 128], f32, tag="xps")
                    nc.tensor.matmul(
                        out=x_ps[:], lhsT=ohc[:, t % OHC, :],
                        rhs=ci_loc[:, lblk, :], start=True, stop=False,
                    )
                    nc.tensor.matmul(
                        out=x_ps[:], lhsT=sT[:], rhs=wls_sb[:],
                        start=False, stop=True,
                    )

                    if t % 4 == 0:
                        mvw = sm.tile([128, 8], f32, tag="mvw", bufs=4)
                        stdw = sm.tile([128, 4], f32, tag="stdw", bufs=4)
                        rstdw = sm.tile([128, 4], f32, tag="rstdw", bufs=4)
                        xsbw = sm.tile([128, 4, 128], f16, tag="xsbw", bufs=3)
                        cjew = [None] * 4
                        ob = w2.tile([128, 4, 128], f32, tag="ob", bufs=4)
                    j4 = t % 4
                    cjew[j4] = cje
                    nc.vector.tensor_tensor(
                        out=xsbw[:, j4, :], in0=cje[:, 0:128], in1=x_ps[:],
                        op=OP.add,
                    )
                    st6 = sm.tile([128, 6], f32, tag="st6")
                    nc.vector.bn_stats(out=st6[:], in_=xsbw[:, j4, :])
                    nc.vector.bn_aggr(out=mvw[:, 2 * j4:2 * j4 + 2], in_=st6[:])
                    if j4 == 3:
                        varv = bass.AP(
                            tensor=mvw.tensor, offset=mvw[:].offset + 1,
                            ap=[mvw[:].ap[0], [2, 4]],
                        )
                        muv = bass.AP(
                            tensor=mvw.tensor, offset=mvw[:].offset,
                            ap=[mvw[:].ap[0], [2, 4]],
                        )
                        nc.scalar.activation(
                            stdw[:], varv, AF.Sqrt, bias=eps_sb[:]
                        )
                        nc.vector.reciprocal(rstdw[:], stdw[:])
                        nmrw = sm.tile([128, 4], f32, tag="nmrw", bufs=4)
                        nc.vector.tensor_scalar(
                            out=nmrw[:], in0=muv, scalar1=-1.0,
                            scalar2=None, op0=OP.mult,
                        )
                        nc.vector.tensor_tensor(
                            out=nmrw[:], in0=nmrw[:], in1=rstdw[:], op=OP.mult
                        )
                        for j in range(4):
                            cj_j = cjew[j]
                            xn = sm.tile([128, 128], f16, tag="xn")
                            nc.scalar.activation(
                                xn[:], xsbw[:, j, :], AF.Identity,
                                bias=nmrw[:, j:j + 1], scale=rstdw[:, j:j + 1],
                            )
                            xf = sm.tile([128, 128], f16, tag="xf")
                            nc.vector.tensor_tensor(
                                out=xf[:], in0=xn[:], in1=cj_j[:, 128:256],
                                op=OP.mult,
                            )
                            xb = sm.tile([128, 128], f16, tag="xb")
                            nc.vector.tensor_tensor(
                                out=xb[:], in0=xf[:], in1=cj_j[:, 256:384],
                                op=OP.add,
                            )
                            if j % 2 == 0:
                                nc.scalar.activation(
                                    ob[:, j, :], xb[:], AF.Relu
                                )
                            else:
                                nc.vector.tensor_scalar_max(
                                    ob[:, j, :], xb[:], 0.0
                                )
                    if t % 4 == 3:
                        e0 = (t - 3) * TILE
                        nc.sync.dma_start(
                            out=out.ap()[e0:e0 + 4 * TILE, :].rearrange(
                                "(t p) d -> p t d", p=128
                            ),
                            in_=ob[:],
                        )

    nc.compile()
    return nc


def _run(inputs, trace=False):
    _axon_shim()
    from concourse.bass_utils import run_bass_kernel_spmd

    ii = np.asarray(inputs["node_i_ids"])
    jj = np.asarray(inputs["node_j_ids"])
    key = hash((ii.tobytes(), jj.tobytes()))
    if _cache.get("key") != key:
        plans, sched = _plan(inputs)
        _cache.update(
            key=key, plans=plans, sched=sched, nc=_build_program(sched)
        )
    plans, sched, nc = _cache["plans"], _cache["sched"], _cache["nc"]
    in_maps = _prep_inputs(inputs, plans, sched)

    res = run_bass_kernel_spmd(
        nc, in_maps, core_ids=list(range(NCORES)), trace=trace
    )
    full = np.zeros((E, 128), np.float32)
    for c, p in enumerate(plans):
        sm_ = p["slotmap"]
        valid = sm_ >= 0
        full[sm_[valid]] = res.results[c]["out"][valid]
    return full, res


def kernel(**inputs):
    full, _ = _run(inputs, trace=False)
    return full.astype(np.float32)

